# revision 12
# baseline (speedup 1.0000x reference)
"""Multi-head attention kernel for Trainium2, sharded over (batch, head-group)
across 8 NeuronCores.

Problem (hardcoded): B=4, N=2048, DIM=1024, NHEADS=16, HEAD_DIM=64.
  q/k/v = x @ W.T + b ; per-head attn = softmax(q k^T / 8) ; raw-reshape
  concat ; out = X @ Wo.T + bo.

Key fact exploited: the reference's "raw reshape" of [(b h), n, d] ->
[b, n, c] makes output rows h*128+i depend ONLY on head h, so head-sharding
needs no collective at the output projection.

Sharding: core c handles batch b=c//2 and heads (c%2)*8 .. +8, producing
output rows (c%2)*1024 .. +1024 of batch b.

Per-core schedule (ACT exp is the binding engine at ~66us/head-pair; the
whole program is software-pipelined so ACT never starves):
  q-proj jt0, k-proj jt0
  for jt in 0..3:
    attention(jt):  per m-chunk: [v-proj(jt,mc) in first n-chunk only],
      S^T pair (row-packed 64x matmuls), exp on ACT (scale folded),
      denominator partial sums on DVE, O^T pair (col-packed, PSUM-accum)
      -> ones-matmul denominator fold, reciprocal, normalize
    q-proj jt+1, k-proj jt+1        (in PE slack under ACT)
    out-proj jt (row-packed), +bo, DMA out
V is projected straight into natural [m, d] layout on the PE (no DMA
transposes). PSUM: s(2x2) + o(2) + shared proj/vproj/outproj (2x1) = 8 banks.
"""

import numpy as np

B = 4
N = 2048
DIM = 1024
NHEADS = 16
HEAD_DIM = 64
SCALE = HEAD_DIM ** -0.5
NCORES = 8
HEADS_PER_CORE = 8  # 4 pairs
JT = 4  # head-pairs per core (j-tiles of 128 features)
MC = 16  # m-chunks of 128
NB = 2  # n-chunks of 1024
NCHUNK = 1024

_CACHE = {}


def _build_program(reps=1, phases="all", gp_split=False):
    import concourse.bass as bass
    import concourse.mybir as mybir
    from concourse import bacc
    from concourse.tile import TileContext

    fp32 = mybir.dt.float32
    bf16 = mybir.dt.bfloat16
    EXP = mybir.ActivationFunctionType.Exp

    nc = bacc.Bacc(None)

    xq = nc.dram_tensor("xq_t", [DIM, N], bf16, kind="ExternalInput")
    xk = nc.dram_tensor("xk_t", [DIM, N], bf16, kind="ExternalInput")
    xv = nc.dram_tensor("xv_t", [DIM, N], bf16, kind="ExternalInput")
    wq = nc.dram_tensor("wq_t", [DIM, 512], bf16, kind="ExternalInput")
    wk = nc.dram_tensor("wk_t", [DIM, 512], bf16, kind="ExternalInput")
    wv = nc.dram_tensor("wv_t", [DIM, 512], bf16, kind="ExternalInput")
    bqkv = nc.dram_tensor("bqkv", [128, 12], fp32, kind="ExternalInput")
    bvr = nc.dram_tensor("bv_rep", [128, 512], fp32, kind="ExternalInput")
    wo = nc.dram_tensor("wo_dup", [128, 16, DIM], bf16, kind="ExternalInput")
    bo_r = nc.dram_tensor("bo_rep", [128, DIM], fp32, kind="ExternalInput")
    out = nc.dram_tensor("out", [HEADS_PER_CORE * 128, DIM], fp32,
                         kind="ExternalOutput")

    xq_v = xq.rearrange("(c p) n -> p c n", p=128)
    xk_v = xk.rearrange("(c p) n -> p c n", p=128)
    xv_v = xv.rearrange("(c p) n -> p c n", p=128)
    wq_v = wq.rearrange("(c p) j -> p c j", p=128)
    wk_v = wk.rearrange("(c p) j -> p c j", p=128)
    wv_v = wv.rearrange("(c p) j -> p c j", p=128)

    with TileContext(nc) as tc:
      for _rep in range(reps):
        with (
            tc.tile_pool(name="persist", bufs=1) as pers,
            tc.tile_pool(name="qkvt", bufs=1) as qkv_pool,
            tc.tile_pool(name="consts", bufs=1) as cpool,
            tc.tile_pool(name="wpool", bufs=1) as wpool,
            tc.tile_pool(name="xt", bufs=4) as xt_pool,
            tc.tile_pool(name="s_psum", bufs=2, space="PSUM") as sp,
            tc.tile_pool(name="o_psum", bufs=1, space="PSUM") as op,
            tc.tile_pool(name="pmisc_psum", bufs=2, space="PSUM") as pm,
            tc.tile_pool(name="epool", bufs=4) as epool,
            tc.tile_pool(name="tpool", bufs=1) as tpool,
            tc.tile_pool(name="rpool", bufs=1) as rpool,
            tc.tile_pool(name="onorm", bufs=2) as onpool,
            tc.tile_pool(name="outsb", bufs=2) as outsb_pool,
        ):
            # ---- constants / weights ----
            b_sb = cpool.tile([128, 12], fp32)  # cols: q jt0..3, k jt0..3
            nc.sync.dma_start(b_sb[:], bqkv[:])
            bvn = cpool.tile([128, 512], fp32)
            nc.sync.dma_start(bvn[:], bvr[:])
            bo_sb = cpool.tile([128, DIM], fp32)
            nc.sync.dma_start(bo_sb[:], bo_r[:])
            ones64 = cpool.tile([128, 64], bf16)
            nc.vector.memset(ones64[:], 1.0)
            w_sbs = []
            for name, wv_ap in (("wq", wq_v), ("wk", wk_v), ("wv", wv_v)):
                t = wpool.tile([128, 8, 512], bf16, tag=name, name=name)
                nc.sync.dma_start(t[:], wv_ap[:])
                w_sbs.append(t)

            # ---- activation loads: 2 batched DMAs per tensor, shared pool ----
            xts_q, xts_k, xts_v = [], [], []
            for x_v, dst in ((xq_v, xts_q), (xk_v, xts_k), (xv_v, xts_v)):
                for g in range(2):
                    xt = xt_pool.tile([128, 4, N], bf16, tag="xt")
                    nc.sync.dma_start(xt[:], x_v[:, g * 4:(g + 1) * 4, :])
                    dst.append(xt)
            wo_sb = pers.tile([128, 16, DIM], bf16)
            nc.sync.dma_start(wo_sb[:], wo[:])

            qT = [qkv_pool.tile([128, N], bf16, tag=f"qT{j}", name=f"qT{j}")
                  for j in range(JT)]
            kT = [qkv_pool.tile([128, N], bf16, tag=f"kT{j}", name=f"kT{j}")
                  for j in range(JT)]
            v_nat = [qkv_pool.tile([128, MC, 128], bf16, tag=f"vn{j}",
                                   name=f"vn{j}") for j in range(JT)]

            def proj(jt, xts, w_sb, dest, bcol):
                # dest[jt] [128 j, 2048 n] = (W x)^T in 4 PSUM quarters
                for q4 in range(4):
                    ps = pm.tile([128, 512], fp32, tag="pmisc")
                    for i in range(8):
                        nc.tensor.matmul(
                            ps[:], w_sb[:, i, jt * 128:(jt + 1) * 128],
                            xts[i // 4][:, i % 4, q4 * 512:(q4 + 1) * 512],
                            start=(i == 0), stop=(i == 7),
                        )
                    nc.vector.tensor_scalar_add(
                        dest[:, q4 * 512:(q4 + 1) * 512], ps[:],
                        b_sb[:, bcol:bcol + 1],
                    )

            def vproj(jt, mc):
                # v_nat[jt][:, mc, :] [128 m, 128 d] natural layout on PE
                ps = pm.tile([128, 512], fp32, tag="pmisc")
                for i in range(8):
                    nc.tensor.matmul(
                        ps[:, :128],
                        xts_v[i // 4][:, i % 4, mc * 128:(mc + 1) * 128],
                        w_sbs[2][:, i, jt * 128:(jt + 1) * 128],
                        start=(i == 0), stop=(i == 7),
                    )
                nc.vector.tensor_add(
                    v_nat[jt][:, mc, :], ps[:, :128],
                    bvn[:, jt * 128:(jt + 1) * 128],
                )

            def attention(jt):
                o_norm = onpool.tile([128, N], bf16, tag="onorm")
                for nb in range(NB):
                    nsl = slice(nb * NCHUNK, (nb + 1) * NCHUNK)
                    o01 = op.tile([128, NCHUNK], fp32, tag="o")
                    T0 = tpool.tile([128, NCHUNK], bf16, tag="T0")
                    T1 = tpool.tile([128, NCHUNK], bf16, tag="T1")

                    def drain(mc, e0, e1):
                        # softmax-denominator partials + O^T accumulation,
                        # issued one m-chunk late so the PE never head-blocks
                        # on the exp that produces e0/e1
                        if mc == 0:
                            nc.vector.tensor_copy(T0[:], e0[:])
                            nc.vector.tensor_copy(T1[:], e1[:])
                        else:
                            nc.vector.tensor_add(T0[:], T0[:], e0[:])
                            nc.vector.tensor_add(T1[:], T1[:], e1[:])
                        for h, e in ((0, e0), (1, e1)):
                            for ns in range(2):
                                nc.tensor.matmul(
                                    o01[h * 64:h * 64 + 64,
                                        ns * 512:(ns + 1) * 512],
                                    v_nat[jt][:, mc, h * 64:h * 64 + 64],
                                    e[:, ns * 512:(ns + 1) * 512],
                                    start=(mc == 0), stop=(mc == MC - 1),
                                    tile_position=(0, h * 64),
                                    skip_group_check=True,
                                )

                    pend = None
                    for mc in range(MC):
                        msl = slice(mc * 128, (mc + 1) * 128)
                        s0 = sp.tile([128, NCHUNK], fp32, tag="s")
                        s1 = sp.tile([128, NCHUNK], fp32, tag="s")
                        for h, s in ((0, s0), (1, s1)):
                            psl = slice(h * 64, h * 64 + 64)
                            for ns in range(2):
                                q_ap = qT[jt][psl,
                                              nb * NCHUNK + ns * 512:
                                              nb * NCHUNK + (ns + 1) * 512]
                                nc.tensor.matmul(
                                    s[:, ns * 512:(ns + 1) * 512],
                                    kT[jt][psl, msl], q_ap,
                                    start=True, stop=True,
                                    tile_position=(h * 64, 0),
                                )
                        if nb == 0:
                            vproj(jt, mc)
                        e0 = epool.tile([128, NCHUNK], bf16, tag="e")
                        e1 = epool.tile([128, NCHUNK], bf16, tag="e")
                        nc.scalar.activation(e0[:], s0[:], EXP, scale=SCALE)
                        nc.scalar.activation(e1[:], s1[:], EXP, scale=SCALE)
                        if pend is not None:
                            drain(*pend)
                        pend = (mc, e0, e1)
                    drain(*pend)
                    # denominator (replicated 64x) via ones-matmul over the
                    # DVE partial sums, then reciprocal + normalize
                    dnrep = sp.tile([128, NCHUNK], fp32, tag="s")
                    for h, Tp in ((0, T0), (1, T1)):
                        for ns in range(2):
                            nc.tensor.matmul(
                                dnrep[h * 64:h * 64 + 64,
                                      ns * 512:(ns + 1) * 512],
                                ones64[:],
                                Tp[:, ns * 512:(ns + 1) * 512],
                                start=True, stop=True,
                                tile_position=(0, h * 64),
                                skip_group_check=True,
                            )
                    rec = rpool.tile([128, NCHUNK], fp32, tag="rec")
                    nc.vector.reciprocal(rec[:], dnrep[:])
                    nc.vector.tensor_mul(o_norm[:, nsl], o01[:], rec[:])
                return o_norm

            def outproj(jt, o_norm):
                on_v = o_norm.rearrange("p (i k) -> p i k", k=16)
                for hh in range(2):
                    base = hh * 64
                    hl = jt * 2 + hh
                    for half in range(2):
                        csl = slice(half * 512, (half + 1) * 512)
                        ops = pm.tile([128, 512], fp32, tag="pmisc")
                        for n2 in range(16):
                            nc.tensor.matmul(
                                ops[:],
                                on_v[base:base + 64, :, n2],
                                wo_sb[base:base + 64, n2, csl],
                                start=(n2 == 0), stop=(n2 == 15),
                                tile_position=(base, 0),
                                skip_group_check=True,
                            )
                        osb = outsb_pool.tile([128, 512], fp32, tag="osb")
                        nc.vector.tensor_add(osb[:], ops[:], bo_sb[:, csl])
                        nc.sync.dma_start(
                            out[hl * 128:(hl + 1) * 128, csl], osb[:]
                        )

            # ---- software-pipelined schedule ----
            for jt in range(JT):
                proj(jt, xts_q, w_sbs[0], qT[jt], jt)
            proj(0, xts_k, w_sbs[1], kT[0], 4)
            for jt in range(JT):
                o_norm = attention(jt)
                if jt + 1 < JT:
                    proj(jt + 1, xts_k, w_sbs[1], kT[jt + 1], 4 + jt + 1)
                outproj(jt, o_norm)

    nc.finalize()
    return nc


def _host_prep(query, key, value, Wq, bq, Wk, bk, Wv, bv, Wo, bo):
    import ml_dtypes

    bf = ml_dtypes.bfloat16
    # Wo.T arranged [16 n2, 64 d, 1024 c'], duplicated along d to 128 partitions
    wot = np.ascontiguousarray(Wo.T).reshape(16, 64, DIM)
    wo_dup = np.ascontiguousarray(
        np.concatenate([wot, wot], axis=1).transpose(1, 0, 2)
    ).astype(bf)
    bo_rep = np.ascontiguousarray(np.broadcast_to(bo, (128, DIM))).astype(np.float32)

    in_maps = []
    for c in range(NCORES):
        b = c // 2
        j0 = (c % 2) * 512
        bias = np.stack(
            [bq[j0:j0 + 512].reshape(4, 128), bk[j0:j0 + 512].reshape(4, 128),
             bv[j0:j0 + 512].reshape(4, 128)], axis=0
        ).reshape(12, 128).T  # [128, 12]
        bv_rep = np.ascontiguousarray(
            np.broadcast_to(bv[j0:j0 + 512], (128, 512))).astype(np.float32)
        in_maps.append({
            "xq_t": np.ascontiguousarray(query[b].T).astype(bf),
            "xk_t": np.ascontiguousarray(key[b].T).astype(bf),
            "xv_t": np.ascontiguousarray(value[b].T).astype(bf),
            "wq_t": np.ascontiguousarray(Wq[j0:j0 + 512].T).astype(bf),
            "wk_t": np.ascontiguousarray(Wk[j0:j0 + 512].T).astype(bf),
            "wv_t": np.ascontiguousarray(Wv[j0:j0 + 512].T).astype(bf),
            "bqkv": np.ascontiguousarray(bias).astype(np.float32),
            "bv_rep": bv_rep,
            "wo_dup": wo_dup,
            "bo_rep": bo_rep,
        })
    return in_maps


def kernel(query, key, value, Wq, bq, Wk, bk, Wv, bv, Wo, bo):
    from concourse.bass_utils import run_bass_kernel_spmd

    query = np.asarray(query, np.float32)
    key = np.asarray(key, np.float32)
    value = np.asarray(value, np.float32)
    in_maps = _host_prep(query, key, value, np.asarray(Wq, np.float32),
                         np.asarray(bq, np.float32), np.asarray(Wk, np.float32),
                         np.asarray(bk, np.float32), np.asarray(Wv, np.float32),
                         np.asarray(bv, np.float32), np.asarray(Wo, np.float32),
                         np.asarray(bo, np.float32))
    if "nc" not in _CACHE:
        _CACHE["nc"] = _build_program()
    res = run_bass_kernel_spmd(_CACHE["nc"], in_maps, core_ids=list(range(NCORES)))
    out = np.empty((B, N, DIM), np.float32)
    for c in range(NCORES):
        b = c // 2
        r0 = (c % 2) * 1024
        out[b, r0:r0 + 1024, :] = res.results[c]["out"]
    return out


if __name__ == "__main__":
    rng = np.random.default_rng(0)
    s = 1.0 / np.sqrt(DIM)
    inp = {
        "query": rng.standard_normal((B, N, DIM), np.float32),
        "key": rng.standard_normal((B, N, DIM), np.float32),
        "value": rng.standard_normal((B, N, DIM), np.float32),
        "Wq": rng.standard_normal((DIM, DIM), np.float32) * s,
        "bq": np.zeros(DIM, np.float32),
        "Wk": rng.standard_normal((DIM, DIM), np.float32) * s,
        "bk": np.zeros(DIM, np.float32),
        "Wv": rng.standard_normal((DIM, DIM), np.float32) * s,
        "bv": np.zeros(DIM, np.float32),
        "Wo": rng.standard_normal((DIM, DIM), np.float32) * s,
        "bo": np.zeros(DIM, np.float32),
    }
    o = kernel(**inp)
    print("ran", o.shape, o.dtype)


# revision 15
# speedup vs baseline: 115.6932x; 115.6932x over previous
"""Multi-head attention kernel for Trainium2, sharded over (batch, head-group)
across 8 NeuronCores.

Problem (hardcoded): B=4, N=2048, DIM=1024, NHEADS=16, HEAD_DIM=64.
  q/k/v = x @ W.T + b ; per-head attn = softmax(q k^T / 8) ; raw-reshape
  concat ; out = X @ Wo.T + bo.

Key fact exploited: the reference's "raw reshape" of [(b h), n, d] ->
[b, n, c] makes output rows h*128+i depend ONLY on head h, so head-sharding
needs no collective at the output projection.

Sharding: core c handles batch b=c//2 and heads (c%2)*8 .. +8, producing
output rows (c%2)*1024 .. +1024 of batch b.

Per-core schedule (ACT exp is the binding engine at ~66us/head-pair; the
whole program is software-pipelined so ACT never starves):
  q-proj jt0, k-proj jt0
  for jt in 0..3:
    attention(jt):  per m-chunk: [v-proj(jt,mc) in first n-chunk only],
      S^T pair (row-packed 64x matmuls), exp on ACT (scale folded),
      denominator partial sums on DVE, O^T pair (col-packed, PSUM-accum)
      -> ones-matmul denominator fold, reciprocal, normalize
    q-proj jt+1, k-proj jt+1        (in PE slack under ACT)
    out-proj jt (row-packed), +bo, DMA out
V is projected straight into natural [m, d] layout on the PE (no DMA
transposes). PSUM: s(2x2) + o(2) + shared proj/vproj/outproj (2x1) = 8 banks.
"""

import numpy as np

B = 4
N = 2048
DIM = 1024
NHEADS = 16
HEAD_DIM = 64
SCALE = HEAD_DIM ** -0.5
NCORES = 8
HEADS_PER_CORE = 8  # 4 pairs
JT = 4  # head-pairs per core (j-tiles of 128 features)
MC = 16  # m-chunks of 128
NB = 2  # n-chunks of 1024
NCHUNK = 1024

_CACHE = {}

# software-pipeline the O^T/denominator drain one m-chunk behind the S/exp
# stream so the PE FIFO never head-blocks on an in-flight exp
O_LAG = True


def _build_program(reps=1, phases="all", gp_split=False):
    import concourse.bass as bass
    import concourse.mybir as mybir
    from concourse import bacc
    from concourse.tile import TileContext

    fp32 = mybir.dt.float32
    bf16 = mybir.dt.bfloat16
    EXP = mybir.ActivationFunctionType.Exp

    nc = bacc.Bacc(None)

    xq = nc.dram_tensor("xq_t", [DIM, N], bf16, kind="ExternalInput")
    xk = nc.dram_tensor("xk_t", [DIM, N], bf16, kind="ExternalInput")
    xv = nc.dram_tensor("xv_t", [DIM, N], bf16, kind="ExternalInput")
    wq = nc.dram_tensor("wq_t", [DIM, 512], bf16, kind="ExternalInput")
    wk = nc.dram_tensor("wk_t", [DIM, 512], bf16, kind="ExternalInput")
    wv = nc.dram_tensor("wv_t", [DIM, 512], bf16, kind="ExternalInput")
    bqkv = nc.dram_tensor("bqkv", [128, 12], fp32, kind="ExternalInput")
    bvr = nc.dram_tensor("bv_rep", [128, 512], fp32, kind="ExternalInput")
    wo = nc.dram_tensor("wo_dup", [128, 16, DIM], bf16, kind="ExternalInput")
    bo_r = nc.dram_tensor("bo_rep", [128, DIM], fp32, kind="ExternalInput")
    out = nc.dram_tensor("out", [HEADS_PER_CORE * 128, DIM], fp32,
                         kind="ExternalOutput")

    xq_v = xq.rearrange("(c p) n -> p c n", p=128)
    xk_v = xk.rearrange("(c p) n -> p c n", p=128)
    xv_v = xv.rearrange("(c p) n -> p c n", p=128)
    wq_v = wq.rearrange("(c p) j -> p c j", p=128)
    wk_v = wk.rearrange("(c p) j -> p c j", p=128)
    wv_v = wv.rearrange("(c p) j -> p c j", p=128)

    with TileContext(nc) as tc:
      for _rep in range(reps):
        with (
            tc.tile_pool(name="persist", bufs=1) as pers,
            tc.tile_pool(name="qkvt", bufs=1) as qkv_pool,
            tc.tile_pool(name="consts", bufs=1) as cpool,
            tc.tile_pool(name="wpool", bufs=1) as wpool,
            tc.tile_pool(name="xt", bufs=4) as xt_pool,
            tc.tile_pool(name="s_psum", bufs=2, space="PSUM") as sp,
            tc.tile_pool(name="o_psum", bufs=1, space="PSUM") as op,
            tc.tile_pool(name="pmisc_psum", bufs=2, space="PSUM") as pm,
            tc.tile_pool(name="epool", bufs=4) as epool,
            tc.tile_pool(name="tpool", bufs=1) as tpool,
            tc.tile_pool(name="rpool", bufs=1) as rpool,
            tc.tile_pool(name="onorm", bufs=2) as onpool,
            tc.tile_pool(name="outsb", bufs=2) as outsb_pool,
        ):
            # ---- constants / weights ----
            b_sb = cpool.tile([128, 12], fp32)  # cols: q jt0..3, k jt0..3
            nc.sync.dma_start(b_sb[:], bqkv[:])
            bvn = cpool.tile([128, 512], fp32)
            nc.sync.dma_start(bvn[:], bvr[:])
            bo_sb = cpool.tile([128, DIM], fp32)
            nc.sync.dma_start(bo_sb[:], bo_r[:])
            ones64 = cpool.tile([128, 64], bf16)
            nc.vector.memset(ones64[:], 1.0)

            # ---- weight + activation loads, interleaved so q-proj can
            # start as soon as (wq, xq) land; x uses 2 batched DMAs per
            # tensor from a shared rotating pool ----
            w_sbs, xts_q, xts_k, xts_v = [], [], [], []
            for (name, wv_ap), (x_v, dst) in zip(
                (("wq", wq_v), ("wk", wk_v), ("wv", wv_v)),
                ((xq_v, xts_q), (xk_v, xts_k), (xv_v, xts_v)),
            ):
                t = wpool.tile([128, 8, 512], bf16, tag=name, name=name)
                nc.sync.dma_start(t[:], wv_ap[:])
                w_sbs.append(t)
                for g in range(2):
                    xt = xt_pool.tile([128, 4, N], bf16, tag="xt")
                    nc.sync.dma_start(xt[:], x_v[:, g * 4:(g + 1) * 4, :])
                    dst.append(xt)
            wo_sb = pers.tile([128, 16, DIM], bf16)
            nc.sync.dma_start(wo_sb[:], wo[:])

            qT = [qkv_pool.tile([128, N], bf16, tag=f"qT{j}", name=f"qT{j}")
                  for j in range(JT)]
            kT = [qkv_pool.tile([128, N], bf16, tag=f"kT{j}", name=f"kT{j}")
                  for j in range(JT)]
            v_nat = [qkv_pool.tile([128, MC, 128], bf16, tag=f"vn{j}",
                                   name=f"vn{j}") for j in range(JT)]

            def proj(jt, xts, w_sb, dest, bcol):
                # dest[jt] [128 j, 2048 n] = (W x)^T in 4 PSUM quarters
                for q4 in range(4):
                    ps = pm.tile([128, 512], fp32, tag="pmisc")
                    for i in range(8):
                        nc.tensor.matmul(
                            ps[:], w_sb[:, i, jt * 128:(jt + 1) * 128],
                            xts[i // 4][:, i % 4, q4 * 512:(q4 + 1) * 512],
                            start=(i == 0), stop=(i == 7),
                        )
                    nc.vector.tensor_scalar_add(
                        dest[:, q4 * 512:(q4 + 1) * 512], ps[:],
                        b_sb[:, bcol:bcol + 1],
                    )

            def vproj(jt, mc):
                # v_nat[jt][:, mc, :] [128 m, 128 d] natural layout on PE
                ps = pm.tile([128, 512], fp32, tag="pmisc")
                for i in range(8):
                    nc.tensor.matmul(
                        ps[:, :128],
                        xts_v[i // 4][:, i % 4, mc * 128:(mc + 1) * 128],
                        w_sbs[2][:, i, jt * 128:(jt + 1) * 128],
                        start=(i == 0), stop=(i == 7),
                    )
                nc.vector.tensor_add(
                    v_nat[jt][:, mc, :], ps[:, :128],
                    bvn[:, jt * 128:(jt + 1) * 128],
                )

            def attention(jt):
                o_norm = onpool.tile([128, N], bf16, tag="onorm")
                for nb in range(NB):
                    nsl = slice(nb * NCHUNK, (nb + 1) * NCHUNK)
                    o01 = op.tile([128, NCHUNK], fp32, tag="o")
                    T0 = tpool.tile([128, NCHUNK], bf16, tag="T0")
                    T1 = tpool.tile([128, NCHUNK], bf16, tag="T1")

                    def drain(mc, e0, e1):
                        # softmax-denominator partials + O^T accumulation,
                        # issued one m-chunk late so the PE never head-blocks
                        # on the exp that produces e0/e1
                        if mc == 0:
                            nc.vector.tensor_copy(T0[:], e0[:])
                            nc.vector.tensor_copy(T1[:], e1[:])
                        else:
                            nc.vector.tensor_add(T0[:], T0[:], e0[:])
                            nc.vector.tensor_add(T1[:], T1[:], e1[:])
                        for h, e in ((0, e0), (1, e1)):
                            for ns in range(2):
                                nc.tensor.matmul(
                                    o01[h * 64:h * 64 + 64,
                                        ns * 512:(ns + 1) * 512],
                                    v_nat[jt][:, mc, h * 64:h * 64 + 64],
                                    e[:, ns * 512:(ns + 1) * 512],
                                    start=(mc == 0), stop=(mc == MC - 1),
                                    tile_position=(0, h * 64),
                                    skip_group_check=True,
                                )

                    pend = None
                    for mc in range(MC):
                        msl = slice(mc * 128, (mc + 1) * 128)
                        s0 = sp.tile([128, NCHUNK], fp32, tag="s")
                        s1 = sp.tile([128, NCHUNK], fp32, tag="s")
                        for h, s in ((0, s0), (1, s1)):
                            psl = slice(h * 64, h * 64 + 64)
                            for ns in range(2):
                                q_ap = qT[jt][psl,
                                              nb * NCHUNK + ns * 512:
                                              nb * NCHUNK + (ns + 1) * 512]
                                nc.tensor.matmul(
                                    s[:, ns * 512:(ns + 1) * 512],
                                    kT[jt][psl, msl], q_ap,
                                    start=True, stop=True,
                                    tile_position=(h * 64, 0),
                                )
                        if nb == 0:
                            vproj(jt, mc)
                        e0 = epool.tile([128, NCHUNK], bf16, tag="e")
                        e1 = epool.tile([128, NCHUNK], bf16, tag="e")
                        nc.scalar.activation(e0[:], s0[:], EXP, scale=SCALE)
                        nc.scalar.activation(e1[:], s1[:], EXP, scale=SCALE)
                        if O_LAG:
                            if pend is not None:
                                drain(*pend)
                            pend = (mc, e0, e1)
                        else:
                            drain(mc, e0, e1)
                    if pend is not None:
                        drain(*pend)
                    # denominator (replicated 64x) via ones-matmul over the
                    # DVE partial sums, then reciprocal + normalize
                    dnrep = sp.tile([128, NCHUNK], fp32, tag="s")
                    for h, Tp in ((0, T0), (1, T1)):
                        for ns in range(2):
                            nc.tensor.matmul(
                                dnrep[h * 64:h * 64 + 64,
                                      ns * 512:(ns + 1) * 512],
                                ones64[:],
                                Tp[:, ns * 512:(ns + 1) * 512],
                                start=True, stop=True,
                                tile_position=(0, h * 64),
                                skip_group_check=True,
                            )
                    rec = rpool.tile([128, NCHUNK], fp32, tag="rec")
                    nc.vector.reciprocal(rec[:], dnrep[:])
                    nc.vector.tensor_mul(o_norm[:, nsl], o01[:], rec[:])
                return o_norm

            def outproj(jt, o_norm):
                on_v = o_norm.rearrange("p (i k) -> p i k", k=16)
                for hh in range(2):
                    base = hh * 64
                    hl = jt * 2 + hh
                    for half in range(2):
                        csl = slice(half * 512, (half + 1) * 512)
                        ops = pm.tile([128, 512], fp32, tag="pmisc")
                        for n2 in range(16):
                            nc.tensor.matmul(
                                ops[:],
                                on_v[base:base + 64, :, n2],
                                wo_sb[base:base + 64, n2, csl],
                                start=(n2 == 0), stop=(n2 == 15),
                                tile_position=(base, 0),
                                skip_group_check=True,
                            )
                        osb = outsb_pool.tile([128, 512], fp32, tag="osb")
                        nc.vector.tensor_add(osb[:], ops[:], bo_sb[:, csl])
                        nc.sync.dma_start(
                            out[hl * 128:(hl + 1) * 128, csl], osb[:]
                        )

            # ---- software-pipelined schedule ----
            for jt in range(JT):
                proj(jt, xts_q, w_sbs[0], qT[jt], jt)
            proj(0, xts_k, w_sbs[1], kT[0], 4)
            for jt in range(JT):
                o_norm = attention(jt)
                if jt + 1 < JT:
                    proj(jt + 1, xts_k, w_sbs[1], kT[jt + 1], 4 + jt + 1)
                outproj(jt, o_norm)

    nc.finalize()
    return nc


def _host_prep(query, key, value, Wq, bq, Wk, bk, Wv, bv, Wo, bo):
    import ml_dtypes

    bf = ml_dtypes.bfloat16
    # Wo.T arranged [16 n2, 64 d, 1024 c'], duplicated along d to 128 partitions
    wot = np.ascontiguousarray(Wo.T).reshape(16, 64, DIM)
    wo_dup = np.ascontiguousarray(
        np.concatenate([wot, wot], axis=1).transpose(1, 0, 2)
    ).astype(bf)
    bo_rep = np.ascontiguousarray(np.broadcast_to(bo, (128, DIM))).astype(np.float32)

    in_maps = []
    for c in range(NCORES):
        b = c // 2
        j0 = (c % 2) * 512
        bias = np.stack(
            [bq[j0:j0 + 512].reshape(4, 128), bk[j0:j0 + 512].reshape(4, 128),
             bv[j0:j0 + 512].reshape(4, 128)], axis=0
        ).reshape(12, 128).T  # [128, 12]
        bv_rep = np.ascontiguousarray(
            np.broadcast_to(bv[j0:j0 + 512], (128, 512))).astype(np.float32)
        in_maps.append({
            "xq_t": np.ascontiguousarray(query[b].T).astype(bf),
            "xk_t": np.ascontiguousarray(key[b].T).astype(bf),
            "xv_t": np.ascontiguousarray(value[b].T).astype(bf),
            "wq_t": np.ascontiguousarray(Wq[j0:j0 + 512].T).astype(bf),
            "wk_t": np.ascontiguousarray(Wk[j0:j0 + 512].T).astype(bf),
            "wv_t": np.ascontiguousarray(Wv[j0:j0 + 512].T).astype(bf),
            "bqkv": np.ascontiguousarray(bias).astype(np.float32),
            "bv_rep": bv_rep,
            "wo_dup": wo_dup,
            "bo_rep": bo_rep,
        })
    return in_maps


def kernel(query, key, value, Wq, bq, Wk, bk, Wv, bv, Wo, bo):
    from concourse.bass_utils import run_bass_kernel_spmd

    query = np.asarray(query, np.float32)
    key = np.asarray(key, np.float32)
    value = np.asarray(value, np.float32)
    in_maps = _host_prep(query, key, value, np.asarray(Wq, np.float32),
                         np.asarray(bq, np.float32), np.asarray(Wk, np.float32),
                         np.asarray(bk, np.float32), np.asarray(Wv, np.float32),
                         np.asarray(bv, np.float32), np.asarray(Wo, np.float32),
                         np.asarray(bo, np.float32))
    if "nc" not in _CACHE:
        _CACHE["nc"] = _build_program()
    res = run_bass_kernel_spmd(_CACHE["nc"], in_maps, core_ids=list(range(NCORES)))
    out = np.empty((B, N, DIM), np.float32)
    for c in range(NCORES):
        b = c // 2
        r0 = (c % 2) * 1024
        out[b, r0:r0 + 1024, :] = res.results[c]["out"]
    return out


if __name__ == "__main__":
    rng = np.random.default_rng(0)
    s = 1.0 / np.sqrt(DIM)
    inp = {
        "query": rng.standard_normal((B, N, DIM), np.float32),
        "key": rng.standard_normal((B, N, DIM), np.float32),
        "value": rng.standard_normal((B, N, DIM), np.float32),
        "Wq": rng.standard_normal((DIM, DIM), np.float32) * s,
        "bq": np.zeros(DIM, np.float32),
        "Wk": rng.standard_normal((DIM, DIM), np.float32) * s,
        "bk": np.zeros(DIM, np.float32),
        "Wv": rng.standard_normal((DIM, DIM), np.float32) * s,
        "bv": np.zeros(DIM, np.float32),
        "Wo": rng.standard_normal((DIM, DIM), np.float32) * s,
        "bo": np.zeros(DIM, np.float32),
    }
    o = kernel(**inp)
    print("ran", o.shape, o.dtype)


# revision 19
# speedup vs baseline: 140.8036x; 1.2170x over previous
"""Multi-head attention kernel for Trainium2, sharded over (batch, head-group)
across 8 NeuronCores.

Problem (hardcoded): B=4, N=2048, DIM=1024, NHEADS=16, HEAD_DIM=64.
  q/k/v = x @ W.T + b ; per-head attn = softmax(q k^T / 8) ; raw-reshape
  concat ; out = X @ Wo.T + bo.

Key fact exploited: the reference's "raw reshape" of [(b h), n, d] ->
[b, n, c] makes output rows h*128+i depend ONLY on head h, so head-sharding
needs no collective at the output projection.

Sharding: core c handles batch b=c//2 and heads (c%2)*8 .. +8, producing
output rows (c%2)*1024 .. +1024 of batch b.

Per-core schedule (ACT exp is the binding engine at ~66us/head-pair; the
whole program is software-pipelined so ACT never starves):
  q-proj jt0, k-proj jt0
  for jt in 0..3:
    attention(jt):  per m-chunk: [v-proj(jt,mc) in first n-chunk only],
      S^T pair (row-packed 64x matmuls), exp on ACT (scale folded),
      denominator partial sums on DVE, O^T pair (col-packed, PSUM-accum)
      -> ones-matmul denominator fold, reciprocal, normalize
    q-proj jt+1, k-proj jt+1        (in PE slack under ACT)
    out-proj jt (row-packed), +bo, DMA out
V is projected straight into natural [m, d] layout on the PE (no DMA
transposes). PSUM: s(2x2) + o(2) + shared proj/vproj/outproj (2x1) = 8 banks.
"""

import numpy as np

B = 4
N = 2048
DIM = 1024
NHEADS = 16
HEAD_DIM = 64
SCALE = HEAD_DIM ** -0.5
NCORES = 8
HEADS_PER_CORE = 8  # 4 pairs
JT = 4  # head-pairs per core (j-tiles of 128 features)
MC = 16  # m-chunks of 128
NB = 2  # n-chunks of 1024
NCHUNK = 1024

_CACHE = {}

# software-pipeline the O^T/denominator drain one m-chunk behind the S/exp
# stream so the PE FIFO never head-blocks on an in-flight exp
O_LAG = True
# project all four head-pairs' V columns per m-chunk (32 N=512 matmuls per
# pair fewer instructions) instead of per-pair N=128 matmuls spread over jt
VPROJ_ALLJT = False


def _build_program(reps=1, phases="all", gp_split=False):
    import concourse.bass as bass
    import concourse.mybir as mybir
    from concourse import bacc
    from concourse.tile import TileContext

    fp32 = mybir.dt.float32
    bf16 = mybir.dt.bfloat16
    EXP = mybir.ActivationFunctionType.Exp

    nc = bacc.Bacc(None)

    xq = nc.dram_tensor("xq_t", [DIM, N], bf16, kind="ExternalInput")
    xk = nc.dram_tensor("xk_t", [DIM, N], bf16, kind="ExternalInput")
    xv = nc.dram_tensor("xv_t", [DIM, N], bf16, kind="ExternalInput")
    wq = nc.dram_tensor("wq_t", [DIM, 512], bf16, kind="ExternalInput")
    wk = nc.dram_tensor("wk_t", [DIM, 512], bf16, kind="ExternalInput")
    wv = nc.dram_tensor("wv_t", [DIM, 512], bf16, kind="ExternalInput")
    bqkv = nc.dram_tensor("bqkv", [128, 12], fp32, kind="ExternalInput")
    bvr = nc.dram_tensor("bv_rep", [128, 512], fp32, kind="ExternalInput")
    wo = nc.dram_tensor("wo_dup", [128, 16, DIM], bf16, kind="ExternalInput")
    bo_r = nc.dram_tensor("bo_rep", [128, DIM], fp32, kind="ExternalInput")
    out = nc.dram_tensor("out", [HEADS_PER_CORE * 128, DIM], fp32,
                         kind="ExternalOutput")

    xq_v = xq.rearrange("(c p) n -> p c n", p=128)
    xk_v = xk.rearrange("(c p) n -> p c n", p=128)
    xv_v = xv.rearrange("(c p) n -> p c n", p=128)
    wq_v = wq.rearrange("(c p) j -> p c j", p=128)
    wk_v = wk.rearrange("(c p) j -> p c j", p=128)
    wv_v = wv.rearrange("(c p) j -> p c j", p=128)

    with TileContext(nc) as tc:
      for _rep in range(reps):
        with (
            tc.tile_pool(name="persist", bufs=1) as pers,
            tc.tile_pool(name="qkvt", bufs=1) as qkv_pool,
            tc.tile_pool(name="consts", bufs=1) as cpool,
            tc.tile_pool(name="wpool", bufs=1) as wpool,
            tc.tile_pool(name="xt", bufs=4) as xt_pool,
            tc.tile_pool(name="s_psum", bufs=2, space="PSUM") as sp,
            tc.tile_pool(name="o_psum", bufs=1, space="PSUM") as op,
            tc.tile_pool(name="pmisc_psum", bufs=2, space="PSUM") as pm,
            tc.tile_pool(name="epool", bufs=4) as epool,
            tc.tile_pool(name="tpool", bufs=1) as tpool,
            tc.tile_pool(name="rpool", bufs=1) as rpool,
            tc.tile_pool(name="onorm", bufs=2) as onpool,
            tc.tile_pool(name="outsb", bufs=2) as outsb_pool,
        ):
            # ---- constants / weights ----
            b_sb = cpool.tile([128, 12], fp32)  # cols: q jt0..3, k jt0..3
            nc.sync.dma_start(b_sb[:], bqkv[:])
            bvn = cpool.tile([128, 512], fp32)
            nc.sync.dma_start(bvn[:], bvr[:])
            bo_sb = cpool.tile([128, DIM], fp32)
            nc.sync.dma_start(bo_sb[:], bo_r[:])
            ones64 = cpool.tile([128, 64], bf16)
            nc.vector.memset(ones64[:], 1.0)

            # ---- weight + activation loads, interleaved so q-proj can
            # start as soon as (wq, xq) land; x uses 2 batched DMAs per
            # tensor from a shared rotating pool ----
            w_sbs, xts_q, xts_k, xts_v = [], [], [], []
            for (name, wv_ap), (x_v, dst) in zip(
                (("wq", wq_v), ("wk", wk_v), ("wv", wv_v)),
                ((xq_v, xts_q), (xk_v, xts_k), (xv_v, xts_v)),
            ):
                t = wpool.tile([128, 8, 512], bf16, tag=name, name=name)
                nc.sync.dma_start(t[:], wv_ap[:])
                w_sbs.append(t)
                for g in range(2):
                    xt = xt_pool.tile([128, 4, N], bf16, tag="xt")
                    nc.sync.dma_start(xt[:], x_v[:, g * 4:(g + 1) * 4, :])
                    dst.append(xt)
            wo_sb = pers.tile([128, 16, DIM], bf16)
            nc.sync.dma_start(wo_sb[:], wo[:])

            qT = [qkv_pool.tile([128, N], bf16, tag=f"qT{j}", name=f"qT{j}")
                  for j in range(JT)]
            kT = [qkv_pool.tile([128, N], bf16, tag=f"kT{j}", name=f"kT{j}")
                  for j in range(JT)]
            if VPROJ_ALLJT:
                v_big = qkv_pool.tile([128, MC, 512], bf16, tag="vn",
                                      name="vn")
                v_nat = [v_big[:, :, j * 128:(j + 1) * 128] for j in range(JT)]
            else:
                v_nat = [qkv_pool.tile([128, MC, 128], bf16, tag=f"vn{j}",
                                       name=f"vn{j}") for j in range(JT)]

            def proj(jt, xts, w_sb, dest, bcol):
                # dest[jt] [128 j, 2048 n] = (W x)^T in 4 PSUM quarters
                for q4 in range(4):
                    ps = pm.tile([128, 512], fp32, tag="pmisc")
                    for i in range(8):
                        nc.tensor.matmul(
                            ps[:], w_sb[:, i, jt * 128:(jt + 1) * 128],
                            xts[i // 4][:, i % 4, q4 * 512:(q4 + 1) * 512],
                            start=(i == 0), stop=(i == 7),
                        )
                    nc.vector.tensor_scalar_add(
                        dest[:, q4 * 512:(q4 + 1) * 512], ps[:],
                        b_sb[:, bcol:bcol + 1],
                    )

            def vproj(jt, mc):
                # v in natural [m, d] layout straight off the PE
                ps = pm.tile([128, 512], fp32, tag="pmisc")
                ncols = 512 if VPROJ_ALLJT else 128
                wsl = (slice(None) if VPROJ_ALLJT
                       else slice(jt * 128, (jt + 1) * 128))
                for i in range(8):
                    nc.tensor.matmul(
                        ps[:, :ncols],
                        xts_v[i // 4][:, i % 4, mc * 128:(mc + 1) * 128],
                        w_sbs[2][:, i, wsl],
                        start=(i == 0), stop=(i == 7),
                    )
                if VPROJ_ALLJT:
                    nc.vector.tensor_add(v_big[:, mc, :], ps[:], bvn[:])
                else:
                    nc.vector.tensor_add(
                        v_nat[jt][:, mc, :], ps[:, :128],
                        bvn[:, jt * 128:(jt + 1) * 128],
                    )

            def attention(jt):
                o_norm = onpool.tile([128, N], bf16, tag="onorm")
                for nb in range(NB):
                    nsl = slice(nb * NCHUNK, (nb + 1) * NCHUNK)
                    o01 = op.tile([128, NCHUNK], fp32, tag="o")
                    T0 = tpool.tile([128, NCHUNK], bf16, tag="T0")
                    T1 = tpool.tile([128, NCHUNK], bf16, tag="T1")

                    def drain(mc, e0, e1):
                        # softmax-denominator partials + O^T accumulation,
                        # issued one m-chunk late so the PE never head-blocks
                        # on the exp that produces e0/e1
                        if mc == 0:
                            nc.vector.tensor_copy(T0[:], e0[:])
                            nc.vector.tensor_copy(T1[:], e1[:])
                        else:
                            nc.vector.tensor_add(T0[:], T0[:], e0[:])
                            nc.vector.tensor_add(T1[:], T1[:], e1[:])
                        for h, e in ((0, e0), (1, e1)):
                            for ns in range(2):
                                nc.tensor.matmul(
                                    o01[h * 64:h * 64 + 64,
                                        ns * 512:(ns + 1) * 512],
                                    v_nat[jt][:, mc, h * 64:h * 64 + 64],
                                    e[:, ns * 512:(ns + 1) * 512],
                                    start=(mc == 0), stop=(mc == MC - 1),
                                    tile_position=(0, h * 64),
                                    skip_group_check=True,
                                )

                    pend = None
                    for mc in range(MC):
                        msl = slice(mc * 128, (mc + 1) * 128)
                        s0 = sp.tile([128, NCHUNK], fp32, tag="s")
                        s1 = sp.tile([128, NCHUNK], fp32, tag="s")
                        for h, s in ((0, s0), (1, s1)):
                            psl = slice(h * 64, h * 64 + 64)
                            for ns in range(2):
                                q_ap = qT[jt][psl,
                                              nb * NCHUNK + ns * 512:
                                              nb * NCHUNK + (ns + 1) * 512]
                                nc.tensor.matmul(
                                    s[:, ns * 512:(ns + 1) * 512],
                                    kT[jt][psl, msl], q_ap,
                                    start=True, stop=True,
                                    tile_position=(h * 64, 0),
                                )
                        if nb == 0 and (jt == 0 or not VPROJ_ALLJT):
                            vproj(jt, mc)
                        e0 = epool.tile([128, NCHUNK], bf16, tag="e")
                        e1 = epool.tile([128, NCHUNK], bf16, tag="e")
                        nc.scalar.activation(e0[:], s0[:], EXP, scale=SCALE)
                        nc.scalar.activation(e1[:], s1[:], EXP, scale=SCALE)
                        if O_LAG:
                            if pend is not None:
                                drain(*pend)
                            pend = (mc, e0, e1)
                        else:
                            drain(mc, e0, e1)
                    if pend is not None:
                        drain(*pend)
                    # denominator (replicated 64x) via ones-matmul over the
                    # DVE partial sums, then reciprocal + normalize
                    dnrep = sp.tile([128, NCHUNK], fp32, tag="s")
                    for h, Tp in ((0, T0), (1, T1)):
                        for ns in range(2):
                            nc.tensor.matmul(
                                dnrep[h * 64:h * 64 + 64,
                                      ns * 512:(ns + 1) * 512],
                                ones64[:],
                                Tp[:, ns * 512:(ns + 1) * 512],
                                start=True, stop=True,
                                tile_position=(0, h * 64),
                                skip_group_check=True,
                            )
                    rec = rpool.tile([128, NCHUNK], fp32, tag="rec")
                    nc.vector.reciprocal(rec[:], dnrep[:])
                    nc.vector.tensor_mul(o_norm[:, nsl], o01[:], rec[:])
                return o_norm

            def outproj(jt, o_norm):
                on_v = o_norm.rearrange("p (i k) -> p i k", k=16)
                for hh in range(2):
                    base = hh * 64
                    hl = jt * 2 + hh
                    for half in range(2):
                        csl = slice(half * 512, (half + 1) * 512)
                        ops = pm.tile([128, 512], fp32, tag="pmisc")
                        for n2 in range(16):
                            nc.tensor.matmul(
                                ops[:],
                                on_v[base:base + 64, :, n2],
                                wo_sb[base:base + 64, n2, csl],
                                start=(n2 == 0), stop=(n2 == 15),
                                tile_position=(base, 0),
                                skip_group_check=True,
                            )
                        osb = outsb_pool.tile([128, 512], fp32, tag="osb")
                        nc.vector.tensor_add(osb[:], ops[:], bo_sb[:, csl])
                        nc.sync.dma_start(
                            out[hl * 128:(hl + 1) * 128, csl], osb[:]
                        )

            # ---- software-pipelined schedule ----
            for jt in range(JT):
                proj(jt, xts_q, w_sbs[0], qT[jt], jt)
            proj(0, xts_k, w_sbs[1], kT[0], 4)
            for jt in range(JT):
                o_norm = attention(jt)
                if jt + 1 < JT:
                    proj(jt + 1, xts_k, w_sbs[1], kT[jt + 1], 4 + jt + 1)
                outproj(jt, o_norm)

    nc.finalize()
    return nc


def _host_prep(query, key, value, Wq, bq, Wk, bk, Wv, bv, Wo, bo):
    import ml_dtypes

    bf = ml_dtypes.bfloat16
    # Wo.T arranged [16 n2, 64 d, 1024 c'], duplicated along d to 128 partitions
    wot = np.ascontiguousarray(Wo.T).reshape(16, 64, DIM)
    wo_dup = np.ascontiguousarray(
        np.concatenate([wot, wot], axis=1).transpose(1, 0, 2)
    ).astype(bf)
    bo_rep = np.ascontiguousarray(np.broadcast_to(bo, (128, DIM))).astype(np.float32)

    in_maps = []
    for c in range(NCORES):
        b = c // 2
        j0 = (c % 2) * 512
        bias = np.stack(
            [bq[j0:j0 + 512].reshape(4, 128), bk[j0:j0 + 512].reshape(4, 128),
             bv[j0:j0 + 512].reshape(4, 128)], axis=0
        ).reshape(12, 128).T  # [128, 12]
        bv_rep = np.ascontiguousarray(
            np.broadcast_to(bv[j0:j0 + 512], (128, 512))).astype(np.float32)
        in_maps.append({
            "xq_t": np.ascontiguousarray(query[b].T).astype(bf),
            "xk_t": np.ascontiguousarray(key[b].T).astype(bf),
            "xv_t": np.ascontiguousarray(value[b].T).astype(bf),
            "wq_t": np.ascontiguousarray(Wq[j0:j0 + 512].T).astype(bf),
            "wk_t": np.ascontiguousarray(Wk[j0:j0 + 512].T).astype(bf),
            "wv_t": np.ascontiguousarray(Wv[j0:j0 + 512].T).astype(bf),
            "bqkv": np.ascontiguousarray(bias).astype(np.float32),
            "bv_rep": bv_rep,
            "wo_dup": wo_dup,
            "bo_rep": bo_rep,
        })
    return in_maps


def kernel(query, key, value, Wq, bq, Wk, bk, Wv, bv, Wo, bo):
    from concourse.bass_utils import run_bass_kernel_spmd

    query = np.asarray(query, np.float32)
    key = np.asarray(key, np.float32)
    value = np.asarray(value, np.float32)
    in_maps = _host_prep(query, key, value, np.asarray(Wq, np.float32),
                         np.asarray(bq, np.float32), np.asarray(Wk, np.float32),
                         np.asarray(bk, np.float32), np.asarray(Wv, np.float32),
                         np.asarray(bv, np.float32), np.asarray(Wo, np.float32),
                         np.asarray(bo, np.float32))
    if "nc" not in _CACHE:
        _CACHE["nc"] = _build_program()
    res = run_bass_kernel_spmd(_CACHE["nc"], in_maps, core_ids=list(range(NCORES)))
    out = np.empty((B, N, DIM), np.float32)
    for c in range(NCORES):
        b = c // 2
        r0 = (c % 2) * 1024
        out[b, r0:r0 + 1024, :] = res.results[c]["out"]
    return out


if __name__ == "__main__":
    rng = np.random.default_rng(0)
    s = 1.0 / np.sqrt(DIM)
    inp = {
        "query": rng.standard_normal((B, N, DIM), np.float32),
        "key": rng.standard_normal((B, N, DIM), np.float32),
        "value": rng.standard_normal((B, N, DIM), np.float32),
        "Wq": rng.standard_normal((DIM, DIM), np.float32) * s,
        "bq": np.zeros(DIM, np.float32),
        "Wk": rng.standard_normal((DIM, DIM), np.float32) * s,
        "bk": np.zeros(DIM, np.float32),
        "Wv": rng.standard_normal((DIM, DIM), np.float32) * s,
        "bv": np.zeros(DIM, np.float32),
        "Wo": rng.standard_normal((DIM, DIM), np.float32) * s,
        "bo": np.zeros(DIM, np.float32),
    }
    o = kernel(**inp)
    print("ran", o.shape, o.dtype)
